# revision 16
# baseline (speedup 1.0000x reference)
"""Trainium2 Bass kernel for nn_ModelConTT_46016279609475 (TT interpolation).

y[b] = v0[b]^T V1[b] V2[b] v3[b], where v_i are linearly-interpolated slices
of tiny TT cores at per-point grid coordinates derived from x[b, :].

Strategy (per NeuronCore, data-parallel over B):
  * The joint pair tables are built ON HOST (weight preprocessing, O(N^2 R^2)
    one-time work independent of B):
      G[n0, n1, k] = sum_c core0[n0, c] * core1[c, n1, k]      (u-side)
      H[n3, n2, k] = sum_c core3[c, n3] * core2[k, n2, c]      (v-side)
    packed 4-corner per cell in f16 (64 useful + 64 pad values = 256 B, the
    minimum legal dma_gather element):
      TAB[(n*128+m), (dn*2+dm)*16 + k] = T[n+dn, m+dm, k]
    so one dma_gather element fetches everything needed for the bilinear
    interpolation of u[b] (and same for v[b]).
  * Device: compute per-point cell ids (int16) + corner weights (f16) from x,
    then 2 dma_gathers per chunk (one per table) and a DVE combine:
      y[b] = sum_k (sum_c wG_c gG[c,k]) * (sum_c wH_c gH[c,k])

Batch mapping per core: shard b of size 32768; on-chip layout is "p-minor":
element i lives at partition i%128, free col i//128, matching dma_gather's
output layout dst[i%128, i//128]. Index lists are mod-16 wrapped as
dma_gather requires (idx for i at [i%16, i//16]) and replicated to rows
16-31 (the queue's core pair); rows 32-127 are zeroed on the Pool engine so
the full-height index AP passes validation.

Pipeline startup: chunk sizes are [8,8] + [16]*14 + [8,8] output columns.
A "mini" index chain (DVE) computes chunks 0-2 from a small duplicated-row
coordinate tile straight into LL rows 0-31 (no marshal DMAs), so the first
gathers launch ~12 us in. The main chain for chunks 3-17 runs on the
otherwise-idle GpSimd engine and its lists are marshalled into wrapped
layout by small per-block DMAs during the first chunks' gather runway.
Small first/last chunks shorten pipeline fill and drain.

Exact-floor trick (f32-safe): t = (xc + 2^23) - 2^23 rounds to nearest;
g = (t > xc); floor = t - g; frac = xc - floor computed via the exact
(t1 - 2^23) path to avoid re-rounding.
"""

import numpy as np

import concourse.bass as bass
import concourse.bacc as bacc
import concourse.mybir as mybir
import concourse.tile as tile
from concourse import library_config
from concourse.bass_utils import run_bass_kernel_spmd

F32 = mybir.dt.float32
F16 = mybir.dt.float16
I16 = mybir.dt.int16
OP = mybir.AluOpType
AF = mybir.ActivationFunctionType

NCORES = 8
B = 262144
BS = B // NCORES          # 32768 points per core
P = 128                   # partitions
J = BS // P               # 256 free cols per partition
JCS = [8, 8] + [16] * 14 + [8, 4, 4]  # output cols per chunk (sum = 256)
NCH = len(JCS)
JOFF = np.cumsum([0] + JCS).tolist()
COFF = [16 * o for o in JOFF]       # int16 list-col offset per chunk
N = 128                   # mode size
R = 16                    # TT rank
TE = N * N                # table entries per side
ES = 128                  # f16 elems per gather element (64 useful + 64 pad)
MAGIC = float(2 ** 23)
SCALE = (N - 1) / 2.0     # 63.5
NB = 8                    # row-blocks in the coordinate tile
CB = 2 * BS // 16 // NB   # 512 pair-cols per row (both sides, per block)
LCOLS = 16 * J            # 4096 total int16 list cols
MINI_CH = 3               # chunks covered by the mini chain (= block 0)

_CACHED = None


def _build_nc():
    nc = bacc.Bacc("TRN2")

    x_pm = nc.dram_tensor("x_pm", [P, J, 4], F32, kind="ExternalInput")
    xq = nc.dram_tensor("xq", [P, CB, 2], F32, kind="ExternalInput")
    xq0 = nc.dram_tensor("xq0", [32, CB, 2], F32, kind="ExternalInput")
    tab = nc.dram_tensor("tab", [2 * TE, ES], F16, kind="ExternalInput")
    y_pm = nc.dram_tensor("y_pm", [P, J], F32, kind="ExternalOutput")

    tabG = tab[0:TE, :]
    tabH = tab[TE : 2 * TE, :]

    with tile.TileContext(nc) as tc:
        # all pools stay open for the whole kernel: a pool close emits a
        # release barrier that would serialize the gather loop behind the
        # entire preamble (SBUF is plentiful here, ~105 KB/partition peak)
        with (
            tc.tile_pool(name="per", bufs=1) as pe,
            tc.tile_pool(name="wp", bufs=1) as wp,
            tc.tile_pool(name="gbuf", bufs=4) as gb,
            tc.tile_pool(name="cbuf", bufs=2) as cb,
        ):
            nc.gpsimd.load_library(library_config.mlp)

            # persistent tiles. LL holds both tables' index lists as flat
            # columns; chunk ch occupies cols [COFF[ch], COFF[ch+1]) with a
            # contiguous G block then an H block.
            LL = pe.tile([P, LCOLS], I16)
            nc.gpsimd.memset(LL[:], 0)
            Wg = pe.tile([P, 4, J], F16)
            Wh = pe.tile([P, 4, J], F16)
            ysb = pe.tile([P, J], F32)
            engs = (nc.sync, nc.scalar)

            # -------- mini index chain: chunks 0-2 straight into LL -------
            # xq0 rows 0-15 hold block-0 coordinate pairs in wrapped list
            # order; rows 16-31 are a host-made duplicate, so one int16
            # store covers LL rows 0-31 with no marshal/replica DMA and the
            # first gathers launch while the main chain still runs.
            xq0_s = wp.tile([32, 2 * CB], F32)
            nc.sync.dma_start(xq0_s[:], xq0[:].rearrange("p a b -> p (a b)"))
            nc.scalar.activation(
                xq0_s[:], xq0_s[:], AF.Copy, bias=SCALE, scale=SCALE
            )
            # floor(xc) == round-to-nearest-even(xc - 0.5) here: xc - 0.5 is
            # exact in f32 over [0, 128) and the only tie (xc == 0 exactly)
            # rounds to 0 == floor. The i16 convert is fused into the op's
            # output dtype.
            lo0 = wp.tile([32, 2 * CB], I16)
            lo0_pair = lo0[:].rearrange("p (m two) -> p m two", two=2)
            c0 = 16 * JCS[0]  # list cols of chunk 0
            # chunk 0 first (tiny ops) so its gathers launch earliest
            nc.vector.tensor_scalar(
                lo0[:, 0 : 2 * c0], xq0_s[:, 0 : 2 * c0], 1.0, -0.5,
                OP.mult, OP.add,
            )
            nc.vector.scalar_tensor_tensor(
                LL[0:32, 0:c0],
                lo0_pair[:, 0:c0, 0],
                128.0,
                lo0_pair[:, 0:c0, 1],
                OP.mult,
                OP.add,
            )
            nc.vector.tensor_scalar(
                lo0[:, 2 * c0 :], xq0_s[:, 2 * c0 :], 1.0, -0.5,
                OP.mult, OP.add,
            )
            nc.vector.scalar_tensor_tensor(
                LL[0:32, c0:CB],
                lo0_pair[:, c0:CB, 0],
                128.0,
                lo0_pair[:, c0:CB, 1],
                OP.mult,
                OP.add,
            )

            # -------- main index chain (chunks 3-17) on GpSimd ----------
            # keeps the DVE free for the mini chain + weights; GpSimd is
            # otherwise idle until descriptor generation ramps.
            xq_s = wp.tile([P, 2 * CB], F32)
            nc.sync.dma_start(xq_s[:], xq[:].rearrange("p a b -> p (a b)"))
            nc.scalar.activation(
                xq_s[:], xq_s[:], AF.Copy, bias=SCALE, scale=SCALE
            )
            loq = wp.tile([P, 2 * CB], I16)
            nc.vector.tensor_scalar(loq[:], xq_s[:], 1.0, -0.5, OP.mult, OP.add)
            # idx = lo_hi*128 + lo_lo (int16 arithmetic, max 16383)
            lo_pair = loq[:].rearrange("p (m two) -> p m two", two=2)
            stage = wp.tile([P, CB], I16)
            nc.vector.scalar_tensor_tensor(
                stage[:],
                lo_pair[:, :, 0],
                128.0,
                lo_pair[:, :, 1],
                OP.mult,
                OP.add,
            )
            # marshal row-blocks 1-7 (chunks 3-17) into LL rows 0-15 and
            # replicate each to rows 16-31, earliest chunks first, DMAs
            # spread across the SP and Act queues.
            for i, rb in enumerate(range(1, NB)):
                cols = slice(CB * rb, CB * (rb + 1))
                engs[i % 2].dma_start(
                    LL[0:16, cols], stage[16 * rb : 16 * rb + 16, :]
                )
                engs[(i + 1) % 2].dma_start(LL[16:32, cols], LL[0:16, cols])

            # ---------------- interp weights (DVE) ----------------
            # x_s is [128, (256 j, 4 d)]; w = frac(xc), a = 1 - w, stored
            # planar f16 in aw = [128, (a|w), 4 d, 256 j] so downstream
            # ops keep stride-1 innermost dims (DVE 2x/4x perf modes).
            x_s = wp.tile([P, J * 4], F32)
            nc.scalar.dma_start(x_s[:], x_pm[:].rearrange("p a b -> p (a b)"))
            nc.scalar.activation(x_s[:], x_s[:], AF.Copy, bias=SCALE, scale=SCALE)
            low = wp.tile([P, J * 4], I16)
            nc.vector.tensor_scalar(low[:], x_s[:], 1.0, -0.5, OP.mult, OP.add)
            lof = wp.tile([P, J * 4], F32)
            nc.vector.tensor_copy(lof[:], low[:])
            aw = wp.tile([P, 2, 4, J], F16)
            # w = xc - floor(xc); write planar-transposed
            xv = x_s[:].rearrange("p (j d) -> p d j", d=4)
            fv = lof[:].rearrange("p (j d) -> p d j", d=4)
            nc.vector.tensor_tensor(aw[:, 1], xv, fv, OP.subtract)
            # a = 1 - w (f16 all through, 4x-eligible)
            nc.vector.tensor_scalar(
                aw[:, 0], aw[:, 1], -1.0, 1.0, OP.mult, OP.add
            )
            # corner products: c order = (hi,lo) in {(a,a),(a,w),(w,a),(w,w)}
            # G uses dims (0 hi, 1 lo); H uses dims (3 hi, 2 lo).
            for W, dhi, dlo in ((Wg, 0, 1), (Wh, 3, 2)):
                in1 = aw[:, :, dlo, :]  # [P, (a|w), J]
                for ci, sel in ((0, 0), (2, 1)):
                    in0 = aw[:, sel, dhi, :].unsqueeze(1).broadcast_to(
                        [P, 2, J]
                    )
                    nc.vector.tensor_tensor(
                        W[:, ci : ci + 2, :], in0, in1, OP.mult
                    )

            # ---------------- gather + combine ----------------
            for ch in range(NCH):
                jc = JCS[ch]
                nidx = P * jc
                gG = gb.tile([P, jc, ES], F16, tag=f"gG{jc}")
                nc.gpsimd.dma_gather(
                    gG[:],
                    tabG,
                    LL[:, COFF[ch] : COFF[ch] + 8 * jc],
                    nidx,
                    nidx,
                    ES,
                    queue_num=0,
                    single_packet=False,
                )
                gH = gb.tile([P, jc, ES], F16, tag=f"gH{jc}")
                nc.gpsimd.dma_gather(
                    gH[:],
                    tabH,
                    LL[:, COFF[ch] + 8 * jc : COFF[ch + 1]],
                    nidx,
                    nidx,
                    ES,
                    queue_num=0,
                    single_packet=False,
                )

                jo = JOFF[ch]
                uv = []
                for ti, (g, W) in enumerate(((gG, Wg), (gH, Wh))):
                    # m[c, j, k] = gathered corner value * corner weight
                    # (weight broadcast over k via stride-0 AP)
                    m = cb.tile([P, 4, jc, R], F16, tag=f"m{ti}{jc}")
                    gv2 = g[:, :, 0 : 4 * R].rearrange(
                        "p j (c k) -> p c j k", k=R
                    )
                    wbc = (
                        W[:, :, jo : jo + jc]
                        .unsqueeze(3)
                        .broadcast_to([P, 4, jc, R])
                    )
                    nc.vector.tensor_tensor(m[:], gv2, wbc, OP.mult)
                    t2 = cb.tile([P, 2, jc, R], F16, tag=f"t{ti}{jc}")
                    nc.vector.tensor_tensor(
                        t2[:], m[:, 0:2], m[:, 2:4], OP.add
                    )
                    u = cb.tile([P, jc, R], F16, tag=f"u{ti}{jc}")
                    nc.vector.tensor_tensor(u[:], t2[:, 0], t2[:, 1], OP.add)
                    uv.append(u)

                pr = cb.tile([P, jc, R], F16, tag=f"pr{jc}")
                nc.vector.tensor_tensor(pr[:], uv[0][:], uv[1][:], OP.mult)
                nc.vector.tensor_reduce(
                    ysb[:, jo : jo + jc],
                    pr[:],
                    mybir.AxisListType.X,
                    OP.add,
                )
                # stream the output back in halves so only the last sliver
                # of writeback sits after the final reduce
                if jo + jc == J // 2:
                    nc.sync.dma_start(y_pm[:, 0 : J // 2], ysb[:, 0 : J // 2])

            nc.sync.dma_start(y_pm[:, J // 2 : J], ysb[:, J // 2 : J])

    nc.finalize()
    return nc


def _pack_table(T):
    """T [128, 128, 16] f32 -> [16384, 128] f16, 4-corner packed + pad."""
    ar = np.arange(N)
    out = np.zeros((N, N, ES), dtype=np.float16)
    for ci, (dn, dm) in enumerate(((0, 0), (0, 1), (1, 0), (1, 1))):
        rn = np.minimum(ar + dn, N - 1)
        rm = np.minimum(ar + dm, N - 1)
        out[:, :, ci * R : ci * R + R] = T[np.ix_(rn, rm)].astype(np.float16)
    return np.ascontiguousarray(out.reshape(TE, ES))


def _prep_inputs(x, core0, core1, core2, core3):
    """Host-side input marshalling: shard x over cores, lay out tensors in
    the on-chip layouts the kernel expects, and precompute the 4-corner
    packed f16 pair tables (weight preprocessing, O(N^2 R^2))."""
    xs = np.ascontiguousarray(np.asarray(x, dtype=np.float32).reshape(NCORES, BS, 4))

    core0 = np.asarray(core0, dtype=np.float32)
    core1 = np.asarray(core1, dtype=np.float32)
    core2 = np.asarray(core2, dtype=np.float32)
    core3 = np.asarray(core3, dtype=np.float32)

    # G[n0, n1, k] = sum_c core0[0, n0, c] core1[c, n1, k]
    G = np.einsum("nc,cmk->nmk", core0[0], core1, optimize=True)
    # H[n3, n2, k] = sum_c core3[c, n3, 0] core2[k, n2, c]
    H = np.einsum("cn,kmc->nmk", core3[:, :, 0], core2, optimize=True)
    tab = np.concatenate([_pack_table(G), _pack_table(H)], axis=0)

    in_maps = []
    for c in range(NCORES):
        xc_ = xs[c]
        x_pm = np.ascontiguousarray(
            xc_.reshape(J, P, 4).transpose(1, 0, 2)
        )  # [128, 256, 4]
        # coordinate pairs in wrapped list order: point i of chunk ch sits
        # at list position [i%16, i//16]; per chunk a G block then H block;
        # chunks concatenated along cols, then rows split into 8 blocks.
        blocks = []
        for ch in range(NCH):
            jc = JCS[ch]
            pts = xc_[128 * JOFF[ch] : 128 * (JOFF[ch] + jc)]
            for dims in ((0, 1), (3, 2)):
                pb = pts[:, dims].reshape(8 * jc, 16, 2).transpose(1, 0, 2)
                blocks.append(pb)  # [16, 8*jc, 2]
        flat = np.concatenate(blocks, axis=1)  # [16, 4096, 2]
        xqa = (
            flat.reshape(16, NB, CB, 2)
            .transpose(1, 0, 2, 3)
            .reshape(NB * 16, CB, 2)
        )  # [128, 512, 2]
        # mini tile: block 0 in rows 0-15, duplicated into rows 16-31
        xq0 = np.concatenate([xqa[0:16], xqa[0:16]], axis=0)  # [32, 512, 2]
        in_maps.append(
            {
                "x_pm": x_pm,
                "xq": np.ascontiguousarray(xqa),
                "xq0": np.ascontiguousarray(xq0),
                "tab": tab,
            }
        )
    return in_maps


def kernel(x, core0, core1, core2, core3):
    global _CACHED
    if _CACHED is None:
        _CACHED = _build_nc()
    nc = _CACHED
    in_maps = _prep_inputs(x, core0, core1, core2, core3)
    res = run_bass_kernel_spmd(nc, in_maps, core_ids=list(range(NCORES)))
    outs = []
    for c in range(NCORES):
        y_pm = res.results[c]["y_pm"]          # [128, 256]
        outs.append(np.ascontiguousarray(np.asarray(y_pm).T).reshape(-1))
    return np.concatenate(outs).astype(np.float32)
